# revision 11
# baseline (speedup 1.0000x reference)
"""Fused multi-head attention (LN + QKV + softmax + out-proj) for TRN2,
sharded over 8 NeuronCores: batch (4) x head-group (2 groups of 6 heads).

Per core, for its (batch, head-group) shard (matmuls bf16, f32 PSUM):
    xn = LayerNorm(x[b])     (gamma+1, 1/sqrt(dh), log2e and x128 folded into Wq)
    xn^T via PE transposes; Q^T,K^T = W @ xn^T pair-packed (even head in
    partitions 0-63, odd head in 64-127); V = xn @ Wv plus a ones column.
    Attention per (head-pair, 1024-query group), kt = 128-key tiles:
      scores: two K=64 row-tiled matmuls (tile_position (0,0)/(64,0)) run the
      even/odd head CONCURRENTLY in the PE array -> two [128,512] PSUM tiles.
      exp: scores are computed in the x128 log2 domain with a -0.5 bias
      (cancels in softmax). Even-head tiles -> ScalarE exp; odd-head tiles ->
      a single fused custom DVE op that emits bf16 BITS via int16 convert
      (floor-frac + quadratic mantissa poly, ~0.5% elem err). This splits the
      N^2 softmax exp across both elementwise engines roughly 50/50.
      PV: attn^T[65, q] += [V|1](kt).T @ P^T per head (row 64 = denominators)
      normalize: fast reciprocal of row 64 (direct from PSUM) + gpsimd
      partition-broadcast + fused multiply-evict into att_sb.
    out partial = attn^T.T @ WoT (bf16, DMA out); host sums the two partials.

PSUM: 4 x [128,512] score slots + 4 x [65,512] PV accumulators = 8 banks.
"""
import numpy as np

import concourse.bass as bass
import concourse.bacc as bacc
import concourse.tile as tile
from concourse import mybir
from concourse.bass_utils import run_bass_kernel_spmd

F32 = mybir.dt.float32
BF16 = mybir.dt.bfloat16
I16 = mybir.dt.int16
AF = mybir.ActivationFunctionType
ALU = mybir.AluOpType

LN2 = 0.6931471805599453
LOG2E = 1.4426950408889634

# ---- fused DVE exp2 -> bf16-bits (single instruction) ----
# Input x = 128*y where y is the log2-domain score (x128 folded into Wq).
# Emits round(x + f*(C1 + C2*f) + C3) as int16 whose bit pattern is the
# bf16 of 2^(y - 0.5). f = x - round128(x) in [-64,64); the -0.5 shift makes
# the round-frac of x the floor-frac of x-64, so the mantissa polynomial is
# smooth over the whole interval (no exponent-borrow kink). The ScalarE path
# applies the same -0.5 via the activation bias; the common 2^-0.5 factor
# cancels in the softmax normalization.
_C0 = 1.5 * 2**30          # rbias: rounds to multiples of 128
_C1 = -0.0066649988252171066
_C2 = 0.0026820964195783934
_C3 = 16181.002631980078    # 16256 - 64 + d0 (+0.5 here if HW convert truncates)

_EXP_OPS = {}


def _register_exp_ops():
    if _EXP_OPS:
        return _EXP_OPS
    from concourse import dve_ops
    from concourse.dve_spec import (Spec, Src0, C0, C1, C2, C3, lower,
                                    _spill_c3_to_src1, _has_src1)
    from concourse.dve_uop import DveOpSpec

    def _ref(in0, in1, c0, c1, c2):
        x = in0.astype(np.float32)
        t = (x + np.float32(c0)).astype(np.float32)
        k = (t - np.float32(c0)).astype(np.float32)
        f = (x - k).astype(np.float32)
        c3 = np.float32(np.asarray(in1, np.float32).reshape(-1)[0])
        return (x + f * (np.float32(c1) + np.float32(c2) * f) + c3).astype(
            np.float32)

    t = Src0 + C0
    k = t - C0
    f = Src0 - k
    body = Src0 + f * (C1 + C2 * f) + C3
    body = _spill_c3_to_src1(body)
    name = "EXP2I16_ATT"
    op = dve_ops.DveOp(name, Spec(body=body, reference=_ref),
                       subdim=False, uops_sha={})
    dve_ops.OPS.append(op)
    dve_ops.CUSTOM_DVE_SPECS[name] = op.spec
    opcode = dve_ops._CUSTOM_DVE_ROW_BASE + len(dve_ops.OPS) - 1
    dve_ops._SUB_OPCODE_FOR_NAME[name] = opcode
    for ver in ("v3", "v4"):
        uops = lower(op.spec, ver=ver)
        op.uops_sha[ver] = DveOpSpec(name=name, opcode=opcode, uops=uops,
                                     rd1_en=_has_src1(op.spec)).sha(ver)
    _EXP_OPS["exp"] = op
    return _EXP_OPS


B, N, DIM, H, DH = 4, 2048, 768, 12, 64
NCORES = 8
NH = 6            # heads per core
NP = 3            # head pairs per core
HCOLS = NH * DH   # 384
QW = 512          # query block width (one PSUM bank per score tile)

# kts (per 16-kt loop) where the odd head's second query-block exp ALSO goes
# to ScalarE instead of the DVE, to balance the two engines.
SE_EXTRA = (5, 11)
SE_ONLY = False   # debug: route every exp tile to ScalarE


def build_graph(n=N, dim=DIM, num_devices=NCORES):
    nt = n // 128        # key tiles
    ncdm = dim // 128    # dmodel chunks

    nc = bacc.Bacc("TRN2", target_bir_lowering=False, debug=False,
                   num_devices=num_devices)
    x = nc.dram_tensor("x", [n, dim], F32, kind="ExternalInput").ap()
    wqt = nc.dram_tensor("wqt", [dim, HCOLS], BF16, kind="ExternalInput").ap()
    wkt = nc.dram_tensor("wkt", [dim, HCOLS], BF16, kind="ExternalInput").ap()
    wvt = nc.dram_tensor("wvt", [dim, HCOLS], BF16, kind="ExternalInput").ap()
    wot = nc.dram_tensor("wot", [HCOLS, dim], BF16, kind="ExternalInput").ap()
    ident = nc.dram_tensor("ident", [128, 128], BF16, kind="ExternalInput").ap()
    out = nc.dram_tensor("out", [n, dim], BF16, kind="ExternalOutput").ap()

    with tile.TileContext(nc) as tc:
        _body(tc, x, wqt, wkt, wvt, wot, ident, out, n, dim, nt, ncdm)
    nc.compile()
    return nc


def _body(tc, x, wqt, wkt, wvt, wot, ident, out, n, dim, nt, ncdm):
    nc = tc.nc
    eo = _register_exp_ops()
    from contextlib import ExitStack
    with ExitStack() as ctx:
        consts = ctx.enter_context(tc.tile_pool(name="consts", bufs=1))
        sb = ctx.enter_context(tc.tile_pool(name="sb", bufs=1))
        xpool = ctx.enter_context(tc.tile_pool(name="xp", bufs=4))
        small = ctx.enter_context(tc.tile_pool(name="small", bufs=4))
        ppool = ctx.enter_context(tc.tile_pool(name="pp", bufs=8))
        rbpool = ctx.enter_context(tc.tile_pool(name="rb", bufs=3))
        oddp = ctx.enter_context(tc.tile_pool(name="odd", bufs=2))
        otp = ctx.enter_context(tc.tile_pool(name="ot", bufs=4))

        # ---- input DMAs: x tiles first (LayerNorm starts ASAP), then weights
        x3 = x.rearrange("(t p) d -> t p d", p=128)
        out3 = out.rearrange("(t p) d -> t p d", p=128)
        xtiles = []
        for tt in range(nt):
            xt_ = sb.tile([128, dim], F32, tag=f"xf{tt}")
            nc.sync.dma_start(out=xt_, in_=x3[tt])
            xtiles.append(xt_)
        id_sb = consts.tile([128, 128], BF16, tag="id")
        nc.sync.dma_start(out=id_sb, in_=ident)
        wv_sb = consts.tile([128, ncdm, HCOLS], BF16, tag="wv")
        nc.sync.dma_start(out=wv_sb, in_=wvt.rearrange("(c p) m -> p c m", p=128))
        wq_sb = consts.tile([128, ncdm, HCOLS], BF16, tag="wq")
        nc.sync.dma_start(out=wq_sb, in_=wqt.rearrange("(c p) m -> p c m", p=128))
        wk_sb = consts.tile([128, ncdm, HCOLS], BF16, tag="wk")
        nc.sync.dma_start(out=wk_sb, in_=wkt.rearrange("(c p) m -> p c m", p=128))
        wo_sb = consts.tile([128, NP, dim], BF16, tag="wo")
        nc.sync.dma_start(out=wo_sb, in_=wot.rearrange("(c p) m -> p c m", p=128))

        # constants
        eps_sb = consts.tile([128, 1], F32, tag="eps")
        nc.vector.memset(eps_sb, 1e-5)
        ebias_sb = consts.tile([128, 1], F32, tag="ebias")
        nc.vector.memset(ebias_sb, -0.5 * LN2)
        c3_sb = consts.tile([128, 1], F32, tag="c3")
        nc.vector.memset(c3_sb, _C3)

        # persistent activations (pair-packed: even head partitions 0-63,
        # odd head 64-127; scores contract K=64 per head via row tiling)
        xnT = sb.tile([128, ncdm, n], BF16, tag="xnT")
        qt_sb = sb.tile([128, NP, n], BF16, tag="qt")
        kt_sb = sb.tile([128, NP, n], BF16, tag="kt")
        v_sb = sb.tile([128, NH, nt, DH + 1], BF16, tag="v")
        nc.vector.memset(v_sb[:, :, :, DH:DH + 1], 1.0)
        att_sb = sb.tile([128, NP, n], BF16, tag="att")

        # ---- phase 1: LayerNorm + transpose + Q/K/V projections ----
        with tc.tile_pool(name="psA", bufs=8, space="PSUM") as psA:
            for tt in range(nt):
                xt = xtiles[tt]
                ngr = dim // 256
                stats = small.tile([128, ngr, 6], F32, tag="stats")
                for g in range(ngr):
                    nc.vector.bn_stats(out=stats[:, g, :],
                                       in_=xt[:, g * 256:(g + 1) * 256])
                mv = small.tile([128, 2], F32, tag="mv")
                nc.vector.bn_aggr(out=mv, in_=stats)
                sq = small.tile([128, 1], F32, tag="sq")
                nc.scalar.activation(out=sq, in_=mv[:, 1:2], func=AF.Sqrt,
                                     bias=eps_sb)
                rstd = small.tile([128, 1], F32, tag="rstd")
                nc.vector.reciprocal(out=rstd, in_=sq)
                xn = xpool.tile([128, dim], BF16, tag="xn")
                nc.vector.tensor_scalar(out=xn, in0=xt, scalar1=mv[:, 0:1],
                                        scalar2=rstd, op0=ALU.subtract,
                                        op1=ALU.mult)
                ptt = psA.tile([128, ncdm, 128], BF16, tag="ptt", bufs=2)
                for c in range(ncdm):
                    nc.tensor.transpose(ptt[:, c, :],
                                        xn[:, c * 128:(c + 1) * 128], id_sb)
                nc.vector.tensor_copy(out=xnT[:, :, tt * 128:(tt + 1) * 128],
                                      in_=ptt)
                # V projection: one 384-wide accumulation chain per tile
                # (stationary xnT chunk loaded once per c)
                pstv = psA.tile([128, HCOLS], F32, tag="pstv", bufs=2)
                for c in range(ncdm):
                    nc.tensor.matmul(pstv,
                                     xnT[:, c, tt * 128:(tt + 1) * 128],
                                     wv_sb[:, c, :],
                                     start=(c == 0), stop=(c == ncdm - 1))
                nc.scalar.copy(out=v_sb[:, :, tt, 0:DH],
                               in_=pstv.rearrange("p (h d) -> p h d", d=DH))
                # Q/K projections per completed 512-token chunk
                if tt % 4 == 3:
                    cc = tt // 4
                    csl = slice(cc * 512, (cc + 1) * 512)
                    for i in range(NP):
                        pst = psA.tile([128, 512], F32, tag="pstq", bufs=4)
                        for c in range(ncdm):
                            nc.tensor.matmul(pst,
                                             wq_sb[:, c, i * 128:(i + 1) * 128],
                                             xnT[:, c, csl],
                                             start=(c == 0), stop=(c == ncdm - 1))
                        nc.scalar.copy(out=qt_sb[:, i, csl], in_=pst)
                        pstk = psA.tile([128, 512], F32, tag="pstq", bufs=4)
                        for c in range(ncdm):
                            nc.tensor.matmul(pstk,
                                             wk_sb[:, c, i * 128:(i + 1) * 128],
                                             xnT[:, c, csl],
                                             start=(c == 0), stop=(c == ncdm - 1))
                        nc.vector.tensor_copy(out=kt_sb[:, i, csl], in_=pstk)

        # ---- phase 2: attention ----
        with tc.tile_pool(name="psS", bufs=4, space="PSUM") as psS, \
             tc.tile_pool(name="psV", bufs=4, space="PSUM") as psV:
            for i in range(NP):
                for qg in range(n // 1024):
                    q0 = qg * 1024
                    # 4 accumulators: (head parity, query 512-block)
                    pvs = [[psV.tile([65, QW], F32, tag="pv",
                                     name=f"pv_{i}_{qg}_{s}_{q}")
                            for q in range(2)] for s in range(2)]
                    for kt in range(nt):
                        ks = slice(kt * 128, (kt + 1) * 128)
                        scs = [[None, None], [None, None]]
                        pts = [[None, None], [None, None]]
                        for q in range(2):
                            qsl = slice(q0 + q * QW, q0 + (q + 1) * QW)
                            # pair-concurrent row-tiled score matmuls
                            for s in range(2):
                                sc = psS.tile([128, QW], F32, tag="sc")
                                rows = slice(s * 64, (s + 1) * 64)
                                nc.tensor.matmul(sc,
                                                 kt_sb[rows, i, ks],
                                                 qt_sb[rows, i, qsl])
                                scs[s][q] = sc
                        for q in range(2):
                            for s in range(2):
                                sc = scs[s][q]
                                use_se = SE_ONLY or (s == 0) or (
                                    q == 1 and (kt % nt) in SE_EXTRA)
                                if use_se:
                                    p_t = ppool.tile([128, QW], BF16, tag="pse")
                                    nc.scalar.activation(out=p_t, in_=sc,
                                                         func=AF.Exp,
                                                         bias=ebias_sb,
                                                         scale=LN2 / 128.0)
                                    pts[s][q] = p_t
                                else:
                                    p_i = ppool.tile([128, QW], I16, tag="pdv")
                                    nc.vector._custom_dve(
                                        eo["exp"], out=p_i, in0=sc,
                                        in1=c3_sb, s0=_C0, s1=_C1, imm2=_C2)
                                    pts[s][q] = p_i[:].bitcast(BF16)
                        for q in range(2):
                            for s in range(2):
                                nc.tensor.matmul(pvs[s][q],
                                                 v_sb[:, 2 * i + s, kt, :],
                                                 pts[s][q],
                                                 start=(kt == 0),
                                                 stop=(kt == nt - 1))
                    # normalize by softmax denominators (row 64)
                    for s in range(2):
                        for q in range(2):
                            pv = pvs[s][q]
                            qsl = slice(q0 + q * QW, q0 + (q + 1) * QW)
                            srow = rbpool.tile([1, QW], F32, tag="sr")
                            nc.vector.tensor_copy(out=srow, in_=pv[64:65, :])
                            rrow = rbpool.tile([1, QW], F32, tag="rr")
                            nc.vector.reciprocal_approx_fast(
                                out=rrow, in_=srow)
                            rc = rbpool.tile([64, QW], F32, tag="rc")
                            nc.gpsimd.partition_broadcast(rc, rrow)
                            if s == 0:
                                nc.vector.tensor_mul(
                                    out=att_sb[0:64, i, qsl],
                                    in0=pv[0:64, :], in1=rc)
                            else:
                                tmp = oddp.tile([64, QW], BF16, tag="odd")
                                nc.vector.tensor_mul(out=tmp, in0=pv[0:64, :],
                                                     in1=rc)
                                nc.sync.dma_start(
                                    out=att_sb[64:128, i, qsl], in_=tmp)

        # ---- phase 3: output projection ----
        with tc.tile_pool(name="psO", bufs=4, space="PSUM") as psO:
            for tt in range(nt):
                po_t = psO.tile([128, dim], F32, tag="psO")
                for c in range(NP):
                    lhsT = att_sb[:, c, tt * 128:(tt + 1) * 128]
                    for o0 in range(0, dim, 512):
                        o1 = min(o0 + 512, dim)
                        nc.tensor.matmul(po_t[:, o0:o1], lhsT,
                                         wo_sb[:, c, o0:o1],
                                         start=(c == 0), stop=(c == NP - 1))
                ot = otp.tile([128, dim], BF16, tag="ot")
                if tt % 2 == 0:
                    nc.vector.tensor_copy(out=ot, in_=po_t)
                else:
                    nc.scalar.copy(out=ot, in_=po_t)
                nc.sync.dma_start(out=out3[tt], in_=ot)


_NC_CACHE = {}


def _get_nc():
    if "nc" not in _NC_CACHE:
        _NC_CACHE["nc"] = build_graph()
    return _NC_CACHE["nc"]


def make_in_maps(x, gamma, Wq, Wk, Wv, Wo):
    """Host-side sharding: core c -> batch c//2, head-group c%2."""
    import ml_dtypes
    bf16 = ml_dtypes.bfloat16
    g = (np.asarray(gamma, np.float32) + 1.0)
    scale = DH ** -0.5 * LOG2E * 128.0  # scores in x128 log2 domain
    Wq_eff = np.asarray(Wq, np.float32) * g[None, :] * scale
    Wk_eff = np.asarray(Wk, np.float32) * g[None, :]
    Wv_eff = np.asarray(Wv, np.float32)
    Wo_eff = np.asarray(Wo, np.float32)
    ident = np.eye(128, dtype=bf16)
    hg_maps = []
    for hg in range(2):
        r0, r1 = hg * HCOLS, (hg + 1) * HCOLS
        hg_maps.append({
            "wqt": np.ascontiguousarray(Wq_eff[r0:r1, :].T).astype(bf16),
            "wkt": np.ascontiguousarray(Wk_eff[r0:r1, :].T).astype(bf16),
            "wvt": np.ascontiguousarray(Wv_eff[r0:r1, :].T).astype(bf16),
            "wot": np.ascontiguousarray(Wo_eff[:, r0:r1].T).astype(bf16),
            "ident": ident,
        })
    in_maps = []
    for c in range(NCORES):
        b, hg = c // 2, c % 2
        m = dict(hg_maps[hg])
        m["x"] = np.ascontiguousarray(np.asarray(x, np.float32)[b])
        in_maps.append(m)
    return in_maps


def _run(inputs, trace=False, trace_kwargs=None):
    nc = _get_nc()
    in_maps = make_in_maps(**inputs)
    res = run_bass_kernel_spmd(nc, in_maps, core_ids=list(range(NCORES)),
                               trace=trace, **(trace_kwargs or {}))
    out = np.empty((B, N, DIM), np.float32)
    for b in range(B):
        out[b] = (res.results[2 * b]["out"].astype(np.float32)
                  + res.results[2 * b + 1]["out"].astype(np.float32))
    return out, res


def kernel(x, gamma, Wq, Wk, Wv, Wo):
    out, _ = _run(dict(x=x, gamma=gamma, Wq=Wq, Wk=Wk, Wv=Wv, Wo=Wo))
    return out


# revision 13
# speedup vs baseline: 1.5029x; 1.5029x over previous
"""Fused multi-head attention (LN + QKV + softmax + out-proj) for TRN2,
sharded over 8 NeuronCores: batch (4) x head-group (2 groups of 6 heads).

Per core, for its (batch, head-group) shard (matmuls bf16, f32 PSUM):
    xn = LayerNorm(x[b])     (gamma+1, 1/sqrt(dh), log2e and x128 folded into Wq)
    xn^T via PE transposes; Q^T,K^T = W @ xn^T pair-packed (even head in
    partitions 0-63, odd head in 64-127); V = xn @ Wv plus a ones column.
    Attention per (head-pair, 1024-query group), kt = 128-key tiles:
      scores: two K=64 row-tiled matmuls (tile_position (0,0)/(64,0)) run the
      even/odd head CONCURRENTLY in the PE array -> two [128,512] PSUM tiles.
      exp: scores are computed in the x128 log2 domain with a -0.5 bias
      (cancels in softmax). Even-head tiles -> ScalarE exp; odd-head tiles ->
      a single fused custom DVE op that emits bf16 BITS via int16 convert
      (floor-frac + quadratic mantissa poly, ~0.5% elem err). This splits the
      N^2 softmax exp across both elementwise engines roughly 50/50.
      PV: attn^T[65, q] += [V|1](kt).T @ P^T per head (row 64 = denominators)
      normalize: fast reciprocal of row 64 (direct from PSUM) + gpsimd
      partition-broadcast + fused multiply-evict into att_sb.
    out partial = attn^T.T @ WoT (bf16, DMA out); host sums the two partials.

PSUM: 4 x [128,512] score slots + 4 x [65,512] PV accumulators = 8 banks.
"""
import numpy as np

import concourse.bass as bass
import concourse.bacc as bacc
import concourse.tile as tile
from concourse import mybir
from concourse.bass_utils import run_bass_kernel_spmd

F32 = mybir.dt.float32
BF16 = mybir.dt.bfloat16
I16 = mybir.dt.int16
AF = mybir.ActivationFunctionType
ALU = mybir.AluOpType

LN2 = 0.6931471805599453
LOG2E = 1.4426950408889634

# ---- fused DVE exp2 -> bf16-bits (single instruction) ----
# Input x = 128*y where y is the log2-domain score (x128 folded into Wq).
# Emits round(x + f*(C1 + C2*f) + C3) as int16 whose bit pattern is the
# bf16 of 2^(y - 0.5). f = x - round128(x) in [-64,64); the -0.5 shift makes
# the round-frac of x the floor-frac of x-64, so the mantissa polynomial is
# smooth over the whole interval (no exponent-borrow kink). The ScalarE path
# applies the same -0.5 via the activation bias; the common 2^-0.5 factor
# cancels in the softmax normalization.
_C0 = 1.5 * 2**30          # rbias: rounds to multiples of 128
_C1 = -0.0066649988252171066
_C2 = 0.0026820964195783934
_C3 = 16181.002631980078    # 16256 - 64 + d0 (+0.5 here if HW convert truncates)

_EXP_OPS = {}


def _register_exp_ops():
    if _EXP_OPS:
        return _EXP_OPS
    from concourse import dve_ops
    from concourse.dve_spec import (Spec, Src0, C0, C1, C2, C3, lower,
                                    _spill_c3_to_src1, _has_src1)
    from concourse.dve_uop import DveOpSpec

    def _ref(in0, in1, c0, c1, c2):
        x = in0.astype(np.float32)
        t = (x + np.float32(c0)).astype(np.float32)
        k = (t - np.float32(c0)).astype(np.float32)
        f = (x - k).astype(np.float32)
        c3 = np.float32(np.asarray(in1, np.float32).reshape(-1)[0])
        return (x + f * (np.float32(c1) + np.float32(c2) * f) + c3).astype(
            np.float32)

    t = Src0 + C0
    k = t - C0
    f = Src0 - k
    body = Src0 + f * (C1 + C2 * f) + C3
    body = _spill_c3_to_src1(body)
    name = "EXP2I16_ATT"
    op = dve_ops.DveOp(name, Spec(body=body, reference=_ref),
                       subdim=False, uops_sha={})
    dve_ops.OPS.append(op)
    dve_ops.CUSTOM_DVE_SPECS[name] = op.spec
    opcode = dve_ops._CUSTOM_DVE_ROW_BASE + len(dve_ops.OPS) - 1
    dve_ops._SUB_OPCODE_FOR_NAME[name] = opcode
    for ver in ("v3", "v4"):
        uops = lower(op.spec, ver=ver)
        op.uops_sha[ver] = DveOpSpec(name=name, opcode=opcode, uops=uops,
                                     rd1_en=_has_src1(op.spec)).sha(ver)
    _EXP_OPS["exp"] = op
    return _EXP_OPS


B, N, DIM, H, DH = 4, 2048, 768, 12, 64
NCORES = 8
NH = 6            # heads per core
NP = 3            # head pairs per core
HCOLS = NH * DH   # 384
QW = 512          # query block width (one PSUM bank per score tile)

# kts (per 16-kt loop) where the odd head's second query-block exp ALSO goes
# to ScalarE instead of the DVE, to balance the two engines.
SE_EXTRA = (5, 11)
SE_ONLY = False   # debug: route every exp tile to ScalarE


def build_graph(n=N, dim=DIM, num_devices=NCORES):
    nt = n // 128        # key tiles
    ncdm = dim // 128    # dmodel chunks

    nc = bacc.Bacc("TRN2", target_bir_lowering=False, debug=False,
                   num_devices=num_devices)
    x = nc.dram_tensor("x", [n, dim], F32, kind="ExternalInput").ap()
    wqt = nc.dram_tensor("wqt", [dim, HCOLS], BF16, kind="ExternalInput").ap()
    wkt = nc.dram_tensor("wkt", [dim, HCOLS], BF16, kind="ExternalInput").ap()
    wvt = nc.dram_tensor("wvt", [dim, HCOLS], BF16, kind="ExternalInput").ap()
    wot = nc.dram_tensor("wot", [HCOLS, dim], BF16, kind="ExternalInput").ap()
    ident = nc.dram_tensor("ident", [128, 128], BF16, kind="ExternalInput").ap()
    out = nc.dram_tensor("out", [n, dim], BF16, kind="ExternalOutput").ap()

    with tile.TileContext(nc) as tc:
        _body(tc, x, wqt, wkt, wvt, wot, ident, out, n, dim, nt, ncdm)
    nc.compile()
    return nc


def _body(tc, x, wqt, wkt, wvt, wot, ident, out, n, dim, nt, ncdm):
    nc = tc.nc
    eo = _register_exp_ops()
    from contextlib import ExitStack
    with ExitStack() as ctx:
        consts = ctx.enter_context(tc.tile_pool(name="consts", bufs=1))
        sb = ctx.enter_context(tc.tile_pool(name="sb", bufs=1))
        xpool = ctx.enter_context(tc.tile_pool(name="xp", bufs=4))
        small = ctx.enter_context(tc.tile_pool(name="small", bufs=4))
        ppool = ctx.enter_context(tc.tile_pool(name="pp", bufs=8))
        rbpool = ctx.enter_context(tc.tile_pool(name="rb", bufs=3))
        oddp = ctx.enter_context(tc.tile_pool(name="odd", bufs=2))
        otp = ctx.enter_context(tc.tile_pool(name="ot", bufs=4))

        # ---- input DMAs: x tiles first (LayerNorm starts ASAP), then weights
        x3 = x.rearrange("(t p) d -> t p d", p=128)
        out3 = out.rearrange("(t p) d -> t p d", p=128)
        # DMA order tuned for the phase-1 critical path: ident (transposes)
        # and wv first, the first x tiles, then wq/wk (needed at token 512),
        # the rest of x, and wo last.
        id_sb = consts.tile([128, 128], BF16, tag="id")
        nc.sync.dma_start(out=id_sb, in_=ident)
        wv_sb = consts.tile([128, ncdm, HCOLS], BF16, tag="wv")
        nc.sync.dma_start(out=wv_sb, in_=wvt.rearrange("(c p) m -> p c m", p=128))
        xtiles = []
        for tt in range(nt):
            xt_ = sb.tile([128, dim], F32, tag=f"xf{tt}")
            xtiles.append(xt_)
        for tt in range(4):
            nc.sync.dma_start(out=xtiles[tt], in_=x3[tt])
        wq_sb = consts.tile([128, ncdm, HCOLS], BF16, tag="wq")
        nc.sync.dma_start(out=wq_sb, in_=wqt.rearrange("(c p) m -> p c m", p=128))
        wk_sb = consts.tile([128, ncdm, HCOLS], BF16, tag="wk")
        nc.sync.dma_start(out=wk_sb, in_=wkt.rearrange("(c p) m -> p c m", p=128))
        for tt in range(4, nt):
            nc.sync.dma_start(out=xtiles[tt], in_=x3[tt])
        wo_sb = consts.tile([128, NP, dim], BF16, tag="wo")
        nc.sync.dma_start(out=wo_sb, in_=wot.rearrange("(c p) m -> p c m", p=128))

        # constants
        eps_sb = consts.tile([128, 1], F32, tag="eps")
        nc.vector.memset(eps_sb, 1e-5)
        ebias_sb = consts.tile([128, 1], F32, tag="ebias")
        nc.vector.memset(ebias_sb, -0.5 * LN2)
        c3_sb = consts.tile([128, 1], F32, tag="c3")
        nc.vector.memset(c3_sb, _C3)

        # persistent activations (pair-packed: even head partitions 0-63,
        # odd head 64-127; scores contract K=64 per head via row tiling)
        xnT = sb.tile([128, ncdm, n], BF16, tag="xnT")
        qt_sb = sb.tile([128, NP, n], BF16, tag="qt")
        kt_sb = sb.tile([128, NP, n], BF16, tag="kt")
        v_sb = sb.tile([128, NH, nt, DH + 1], BF16, tag="v")
        nc.vector.memset(v_sb[:, :, :, DH:DH + 1], 1.0)
        att_sb = sb.tile([128, NP, n], BF16, tag="att")

        # ---- phase 1: LayerNorm + transpose + Q/K/V projections ----
        with tc.tile_pool(name="psA", bufs=8, space="PSUM") as psA:
            for tt in range(nt):
                xt = xtiles[tt]
                ngr = dim // 256
                stats = small.tile([128, ngr, 6], F32, tag="stats")
                for g in range(ngr):
                    nc.vector.bn_stats(out=stats[:, g, :],
                                       in_=xt[:, g * 256:(g + 1) * 256])
                mv = small.tile([128, 2], F32, tag="mv")
                nc.vector.bn_aggr(out=mv, in_=stats)
                sq = small.tile([128, 1], F32, tag="sq")
                nc.scalar.activation(out=sq, in_=mv[:, 1:2], func=AF.Sqrt,
                                     bias=eps_sb)
                rstd = small.tile([128, 1], F32, tag="rstd")
                nc.vector.reciprocal(out=rstd, in_=sq)
                xn = xpool.tile([128, dim], BF16, tag="xn")
                nc.vector.tensor_scalar(out=xn, in0=xt, scalar1=mv[:, 0:1],
                                        scalar2=rstd, op0=ALU.subtract,
                                        op1=ALU.mult)
                ptt = psA.tile([128, ncdm, 128], BF16, tag="ptt", bufs=2)
                for c in range(ncdm):
                    nc.tensor.transpose(ptt[:, c, :],
                                        xn[:, c * 128:(c + 1) * 128], id_sb)
                nc.vector.tensor_copy(out=xnT[:, :, tt * 128:(tt + 1) * 128],
                                      in_=ptt)
                # V projection: one 384-wide accumulation chain per tile
                # (stationary xnT chunk loaded once per c)
                pstv = psA.tile([128, HCOLS], F32, tag="pstv", bufs=2)
                for c in range(ncdm):
                    nc.tensor.matmul(pstv,
                                     xnT[:, c, tt * 128:(tt + 1) * 128],
                                     wv_sb[:, c, :],
                                     start=(c == 0), stop=(c == ncdm - 1))
                nc.scalar.copy(out=v_sb[:, :, tt, 0:DH],
                               in_=pstv.rearrange("p (h d) -> p h d", d=DH))
                # Q/K projections per completed 512-token chunk
                if tt % 4 == 3:
                    cc = tt // 4
                    csl = slice(cc * 512, (cc + 1) * 512)
                    for i in range(NP):
                        pst = psA.tile([128, 512], F32, tag="pstq", bufs=4)
                        for c in range(ncdm):
                            nc.tensor.matmul(pst,
                                             wq_sb[:, c, i * 128:(i + 1) * 128],
                                             xnT[:, c, csl],
                                             start=(c == 0), stop=(c == ncdm - 1))
                        nc.scalar.copy(out=qt_sb[:, i, csl], in_=pst)
                        pstk = psA.tile([128, 512], F32, tag="pstq", bufs=4)
                        for c in range(ncdm):
                            nc.tensor.matmul(pstk,
                                             wk_sb[:, c, i * 128:(i + 1) * 128],
                                             xnT[:, c, csl],
                                             start=(c == 0), stop=(c == ncdm - 1))
                        nc.vector.tensor_copy(out=kt_sb[:, i, csl], in_=pstk)

        # ---- phase 2: attention ----
        # Blocks of (head pair, 512-query). Per kt: two pair-concurrent
        # row-tiled score matmuls -> 2 slots, even exp on ScalarE / odd on
        # DVE, two PV accumulations. 5 score slots (2.5 kt of lookahead so
        # the PE free-runs ahead of the exps) + 3 PV slots = 8 PSUM banks.
        with tc.tile_pool(name="psS", bufs=5, space="PSUM") as psS, \
             tc.tile_pool(name="psV", bufs=3, space="PSUM") as psV:
            for i in range(NP):
                for qh in range(n // QW):
                    q0 = qh * QW
                    qsl = slice(q0, q0 + QW)
                    pvs = [psV.tile([65, QW], F32, tag="pv",
                                    name=f"pv_{i}_{qh}_{s}")
                           for s in range(2)]
                    for kt in range(nt):
                        ks = slice(kt * 128, (kt + 1) * 128)
                        scs = [None, None]
                        pts = [None, None]
                        for s in range(2):
                            sc = psS.tile([128, QW], F32, tag="sc")
                            rows = slice(s * 64, (s + 1) * 64)
                            nc.tensor.matmul(sc, kt_sb[rows, i, ks],
                                             qt_sb[rows, i, qsl])
                            scs[s] = sc
                        for s in range(2):
                            use_se = SE_ONLY or (s == 0) or (
                                (kt + 5 * qh) % 11 == 3)
                            if use_se:
                                p_t = ppool.tile([128, QW], BF16, tag="pse")
                                nc.scalar.activation(out=p_t, in_=scs[s],
                                                     func=AF.Exp,
                                                     bias=ebias_sb,
                                                     scale=LN2 / 128.0)
                                pts[s] = p_t
                            else:
                                p_i = ppool.tile([128, QW], I16, tag="pdv")
                                nc.vector._custom_dve(
                                    eo["exp"], out=p_i, in0=scs[s],
                                    in1=c3_sb, s0=_C0, s1=_C1, imm2=_C2)
                                pts[s] = p_i[:].bitcast(BF16)
                        for s in range(2):
                            nc.tensor.matmul(pvs[s],
                                             v_sb[:, 2 * i + s, kt, :],
                                             pts[s],
                                             start=(kt == 0),
                                             stop=(kt == nt - 1))
                    # normalize by softmax denominators (row 64)
                    for s in range(2):
                        pv = pvs[s]
                        srow = rbpool.tile([1, QW], F32, tag="sr")
                        nc.vector.tensor_copy(out=srow, in_=pv[64:65, :])
                        rrow = rbpool.tile([1, QW], F32, tag="rr")
                        nc.vector.reciprocal_approx_fast(out=rrow, in_=srow)
                        rc = rbpool.tile([64, QW], F32, tag="rc")
                        nc.gpsimd.partition_broadcast(rc, rrow)
                        if s == 0:
                            nc.vector.tensor_mul(out=att_sb[0:64, i, qsl],
                                                 in0=pv[0:64, :], in1=rc)
                        else:
                            tmp = oddp.tile([64, QW], BF16, tag="odd")
                            nc.vector.tensor_mul(out=tmp, in0=pv[0:64, :],
                                                 in1=rc)
                            nc.sync.dma_start(out=att_sb[64:128, i, qsl],
                                              in_=tmp)

        # ---- phase 3: output projection ----
        with tc.tile_pool(name="psO", bufs=4, space="PSUM") as psO:
            for tt in range(nt):
                po_t = psO.tile([128, dim], F32, tag="psO")
                for c in range(NP):
                    lhsT = att_sb[:, c, tt * 128:(tt + 1) * 128]
                    for o0 in range(0, dim, 512):
                        o1 = min(o0 + 512, dim)
                        nc.tensor.matmul(po_t[:, o0:o1], lhsT,
                                         wo_sb[:, c, o0:o1],
                                         start=(c == 0), stop=(c == NP - 1))
                ot = otp.tile([128, dim], BF16, tag="ot")
                if tt % 2 == 0:
                    nc.vector.tensor_copy(out=ot, in_=po_t)
                else:
                    nc.scalar.copy(out=ot, in_=po_t)
                nc.sync.dma_start(out=out3[tt], in_=ot)


_NC_CACHE = {}


def _get_nc():
    if "nc" not in _NC_CACHE:
        _NC_CACHE["nc"] = build_graph()
    return _NC_CACHE["nc"]


def make_in_maps(x, gamma, Wq, Wk, Wv, Wo):
    """Host-side sharding: core c -> batch c//2, head-group c%2."""
    import ml_dtypes
    bf16 = ml_dtypes.bfloat16
    g = (np.asarray(gamma, np.float32) + 1.0)
    scale = DH ** -0.5 * LOG2E * 128.0  # scores in x128 log2 domain
    Wq_eff = np.asarray(Wq, np.float32) * g[None, :] * scale
    Wk_eff = np.asarray(Wk, np.float32) * g[None, :]
    Wv_eff = np.asarray(Wv, np.float32)
    Wo_eff = np.asarray(Wo, np.float32)
    ident = np.eye(128, dtype=bf16)
    hg_maps = []
    for hg in range(2):
        r0, r1 = hg * HCOLS, (hg + 1) * HCOLS
        hg_maps.append({
            "wqt": np.ascontiguousarray(Wq_eff[r0:r1, :].T).astype(bf16),
            "wkt": np.ascontiguousarray(Wk_eff[r0:r1, :].T).astype(bf16),
            "wvt": np.ascontiguousarray(Wv_eff[r0:r1, :].T).astype(bf16),
            "wot": np.ascontiguousarray(Wo_eff[:, r0:r1].T).astype(bf16),
            "ident": ident,
        })
    in_maps = []
    for c in range(NCORES):
        b, hg = c // 2, c % 2
        m = dict(hg_maps[hg])
        m["x"] = np.ascontiguousarray(np.asarray(x, np.float32)[b])
        in_maps.append(m)
    return in_maps


def _run(inputs, trace=False, trace_kwargs=None):
    nc = _get_nc()
    in_maps = make_in_maps(**inputs)
    res = run_bass_kernel_spmd(nc, in_maps, core_ids=list(range(NCORES)),
                               trace=trace, **(trace_kwargs or {}))
    out = np.empty((B, N, DIM), np.float32)
    for b in range(B):
        out[b] = (res.results[2 * b]["out"].astype(np.float32)
                  + res.results[2 * b + 1]["out"].astype(np.float32))
    return out, res


def kernel(x, gamma, Wq, Wk, Wv, Wo):
    out, _ = _run(dict(x=x, gamma=gamma, Wq=Wq, Wk=Wk, Wv=Wv, Wo=Wo))
    return out
